# revision 1
# baseline (speedup 1.0000x reference)
"""Trainium2 Bass kernel for nn_Conv4d (K separate Conv3d layers folded into a
single conv3d with K*Co output channels + temporal accumulation).

Problem (hardcoded):
  x:      [B=2, Ci=8, T=16, D=40, H=40, W=40] f32
  weight: [K=3, Co=32, Ci=8, 3, 3, 3] f32
  bias:   [K=3, Co=32] f32
  out:    [B=2, Co=32, O=16, 40, 40, 40] f32
  out[b, co, o] = sum_k ( conv3d(x[b, :, o+k-1], weight[k], pad=1) + bias[k] )
  with out-of-range temporal frames skipped (zero contribution, incl. bias).

Sharding: data-parallel over the 32 B*T frames, 4 consecutive frames per core.
Each core computes conv3d y[j] = W * x[j] + bias for its 4 frames and
accumulates frame j's three k-blocks into output-frame partials
o = j-1, j, j+1 on-chip; partial sums are combined on the host (adjacent
cores/accumulators overlap at boundaries; addition is exact in fp32).

Device dataflow per core (frames j=0..3, output depth slice d=0..39):
  - XR tile [73, 1680]: partition p = kh*24 + kd*8 + ci holds the contiguous
    40x42 window rows [kh, kh+40) of the padded plane xpad[ci, d+kd]; row 72
    holds ones (bias trick). One DMA per kh: dst [24,1680], src 3D
    (kd, ci, f) dim-reordered AP (DMA lowering allows at most 3 dims and
    requires a single partition dim on the SBUF side). DMA issue is spread
    over the sync/scalar/gpsimd queues to avoid sequencer serialization.
  - 3 kw-matmuls (fp32r, N=400 per h-chunk, 4 chunks) accumulate
    psum[128, 4x512]; M = 128 = 4 blocks of 32 channels. The kw=1 matmul
    includes the 73rd ones-row whose weight row is bias -> bias added exactly
    once per frame j.
  - M-block layouts differ by j parity so that every psum eviction is a
    single partition-base-0 op (the ISA forbids e.g. base 32 + span 64):
    even j: block mb holds k = 2-mb (mb=3 zero weights)
    odd  j: block mb holds k = 3-mb (mb=0 zero weights)
    Then psum block mb always maps to acc block mb of the right accumulator:
    accA [128,1600] holds o_l = 0..3 (o = t0-1+o_l), accB holds o_l = 2..5;
    j=0/2: DVE copy psum[0:128] -> acc (zero block writes a harmless 0)
    j=1/3: DVE add  psum[0:128] into acc (zero block adds 0)
  - DMA accA/accB -> DRAM [128, 40, 1600] per d.
"""

import numpy as np

_STATE: dict = {}

# ---- problem constants --------------------------------------------------
B, CI, T, D, H, W = 2, 8, 16, 40, 40, 40
K, CO = 3, 32
O = 16
NCORES = 8
FRAMES = 4          # frames per core
DP, HP, WP = D + 2, H + 2, W + 2   # padded dims
HW = H * WP          # 40*42 = free size of one (h,w') window
NHC = 4              # h-chunks per d-slice
HCROWS = H // NHC    # 10 rows -> N=400 per matmul


def _build_nc():
    import concourse.mybir as mybir
    from concourse import bacc
    from concourse.tile import TileContext

    f32 = mybir.dt.float32
    f32r = mybir.dt.float32r

    nc = bacc.Bacc(
        "TRN2", target_bir_lowering=False, debug=False, num_devices=NCORES
    )
    xp = nc.dram_tensor("xp", [CI, FRAMES, DP, HP, WP], f32r, kind="ExternalInput")
    wb = nc.dram_tensor("wb", [73, 768], f32r, kind="ExternalInput")
    ones = nc.dram_tensor("ones", [1, HW], f32r, kind="ExternalInput")
    outA = nc.dram_tensor("outA", [128, D, H * W], f32, kind="ExternalOutput")
    outB = nc.dram_tensor("outB", [128, D, H * W], f32, kind="ExternalOutput")

    with TileContext(nc) as tc:
        with (
            tc.tile_pool(name="const", bufs=1) as pc,
            tc.tile_pool(name="xr", bufs=6) as px,
            tc.tile_pool(name="acc", bufs=2) as pa,
            tc.tile_pool(name="ps", bufs=2, space="PSUM") as pp,
        ):
            wbt = pc.tile([73, 768], f32r)
            nc.sync.dma_start(wbt[:, :], wb[:, :])
            for d in range(D):
                accA = pa.tile([128, H * W], f32, tag="accA")
                accB = pa.tile([128, H * W], f32, tag="accB")
                for j in range(FRAMES):
                    xr = px.tile([73, HW], f32r, tag="xr")
                    for kh in range(3):
                        src = xp[:, j, d : d + 3, kh : kh + H, :].rearrange(
                            "ci kd h w -> kd ci (h w)"
                        )
                        nc.gpsimd.dma_start(xr[kh * 24 : (kh + 1) * 24, :], src)
                    nc.scalar.dma_start(xr[72:73, :], ones[:, :])

                    ps = pp.tile([128, 4 * 512], f32, tag="ps")
                    xrv = xr[:, :].rearrange("p (h w) -> p h w", w=WP)
                    par = j % 2
                    for kw in range(3):
                        rows = 73 if kw == 1 else 72
                        lhsT = wbt[0:rows, (par * 3 + kw) * 128 : (par * 3 + kw + 1) * 128]
                        for hc in range(NHC):
                            rhs = xrv[
                                0:rows,
                                hc * HCROWS : (hc + 1) * HCROWS,
                                kw : kw + W,
                            ]
                            nc.tensor.matmul(
                                ps[:, hc * 512 : hc * 512 + HCROWS * W],
                                lhsT,
                                rhs,
                                start=(kw == 0),
                                stop=(kw == 2),
                            )
                    psv = ps[:, :].rearrange("p (b c) -> p b c", c=512)[
                        :, :, 0 : HCROWS * W
                    ]
                    acc = accA if j < 2 else accB
                    accv = acc[:, :].rearrange("p (b c) -> p b c", c=HCROWS * W)
                    if par == 0:
                        nc.vector.tensor_copy(accv, psv)
                    else:
                        nc.vector.tensor_add(accv, psv, accv)
                nc.gpsimd.dma_start(outA[:, d, :], accA[:, :])
                nc.gpsimd.dma_start(outB[:, d, :], accB[:, :])
    nc.compile()
    return nc


def _get_nc():
    if "nc" not in _STATE:
        _STATE["nc"] = _build_nc()
    return _STATE["nc"]


def _host_inputs(x, weight, bias):
    """Build per-core input maps."""
    x = np.ascontiguousarray(x, dtype=np.float32)
    weight = np.ascontiguousarray(weight, dtype=np.float32)
    bias = np.ascontiguousarray(bias, dtype=np.float32)

    # weight [k, co, ci, kd, kh, kw] -> [kh, kd, ci, kw, k'(=2-k), co]
    wrev = weight.transpose(4, 3, 2, 5, 0, 1)[:, :, :, :, ::-1, :]
    # col layout: par*384 + kw*128 + mb*32 + co
    #   par=0 (even j): blocks 0..2 = k reversed, block 3 zero
    #   par=1 (odd  j): block 0 zero, blocks 1..3 = k reversed
    wbh = np.zeros((73, 768), np.float32)
    w_even = np.zeros((3, 3, 8, 3, 4, 32), np.float32)
    w_even[:, :, :, :, 0:3] = wrev
    w_odd = np.zeros((3, 3, 8, 3, 4, 32), np.float32)
    w_odd[:, :, :, :, 1:4] = wrev
    wbh[0:72, 0:384] = w_even.reshape(72, 384)
    wbh[0:72, 384:768] = w_odd.reshape(72, 384)
    brev = bias[::-1].reshape(96)  # bias in kw=1 block, k-reversed
    wbh[72, 128 : 128 + 96] = brev          # even layout, kw=1, blocks 0..2
    wbh[72, 384 + 128 + 32 : 384 + 256] = brev  # odd layout, kw=1, blocks 1..3
    onesh = np.ones((1, HW), np.float32)

    in_maps = []
    for c in range(NCORES):
        b, tb = divmod(c, 4)
        t0 = tb * FRAMES
        xpc = np.zeros((CI, FRAMES, DP, HP, WP), np.float32)
        xpc[:, :, 1 : 1 + D, 1 : 1 + H, 1 : 1 + W] = x[b, :, t0 : t0 + FRAMES]
        in_maps.append({"xp": xpc, "wb": wbh, "ones": onesh})
    return in_maps


def _assemble(results):
    out = np.zeros((B, CO, O, D, H, W), np.float32)
    for c in range(NCORES):
        b, tb = divmod(c, 4)
        t0 = tb * FRAMES
        A = results[c]["outA"].reshape(4, 32, D, H, W)
        Bv = results[c]["outB"].reshape(4, 32, D, H, W)
        for i in range(4):
            o = t0 - 1 + i
            if 0 <= o < O:
                out[b, :, o] += A[i]
            o = t0 + 1 + i
            if 0 <= o < O:
                out[b, :, o] += Bv[i]
    return out


def _run(x, weight, bias, trace=False, tmpdir=None):
    from concourse.bass_utils import run_bass_kernel_spmd

    if trace:
        _install_ntff_hook()
    nc = _get_nc()
    in_maps = _host_inputs(x, weight, bias)
    res = run_bass_kernel_spmd(
        nc,
        in_maps,
        core_ids=list(range(NCORES)),
        trace=trace,
        tmpdir=tmpdir,
    )
    return _assemble(res.results), res.exec_time_ns


def _install_ntff_hook():
    """Register the axon NTFF profile hook (missing from this image's antenv)."""
    import sys, types

    if "antenv.axon_hooks" in sys.modules:
        return
    mod = types.ModuleType("antenv.axon_hooks")
    holder = [None]
    mod.set_axon_ntff_profile_hook = lambda h: holder.__setitem__(0, h)
    mod.get_axon_ntff_profile_hook = lambda: holder[0]
    sys.modules["antenv.axon_hooks"] = mod
    try:
        from trn_agent_boot.trn_boot import _ntff_profile_via_ctypes

        mod.set_axon_ntff_profile_hook(
            _ntff_profile_via_ctypes("/opt/axon/libaxon_pjrt.so")
        )
    except Exception:
        pass


def kernel(x, weight, bias):
    out, _ = _run(x, weight, bias, trace=False)
    return out

